# revision 16
# baseline (speedup 1.0000x reference)
"""Trainium2 Bass kernel for nn_Coembedding (dual-MLP cosine-similarity retrieval).

Computation (see reference):
    mp = relu(molecule @ Wm1.T + bm1) @ Wm2.T + bm2          [N, D]
    pp = relu(protein  @ Wp1.T + bp1) @ Wp2.T + bp2          [M, D]
    out = (pp/|pp| @ (mp/|mp|).T) / temperature              [M, N]

Distribution over 8 NeuronCores:
  - molecule rows (N) sharded 8x for the molecule MLP; normalized embeddings
    (feature-major [D, N/8]) AllGathered so every core holds all N molecule
    embeddings.
  - protein rows (M) sharded 8x; each core computes its own protein MLP shard
    and the [M/8, N] similarity tile.

All on-chip layouts are feature-major (K on partitions) so the two MLP layers
and the similarity GEMM chain without transposes.  Matmuls run as float32r
(full-rate fp32 streaming, ~1e-4 matmul rel-err vs 4x slower exact fp32).
All DRAM layouts are pre-tiled host-side so every DMA is a linear
partition-major copy (large contiguous runs per partition).
"""

import numpy as np
from contextlib import ExitStack

import concourse.bass as bass
import concourse.tile as tile
from concourse import bacc, mybir
from concourse.bass_utils import run_bass_kernel_spmd

F32 = mybir.dt.float32
F32R = mybir.dt.float32r
BF16 = mybir.dt.bfloat16
AF = mybir.ActivationFunctionType

N_CORES = 8
N, M, MOL, PROT, D = 4096, 8192, 768, 1280, 1024
NS = N // N_CORES            # 512 molecule rows per core
MS = M // N_CORES            # 1024 protein rows per core
KM, KP, KD = MOL // 128, PROT // 128, D // 128   # 6, 10, 8 contraction chunks
DC = D // 128                # 8 output-feature chunks
EPS = 1e-8

_CACHE: dict = {}


def _build():
    if "nc" in _CACHE:
        return _CACHE["nc"]

    nc = bacc.Bacc("TRN2", target_bir_lowering=False, debug=False,
                   num_devices=N_CORES)

    # All inputs pre-tiled host-side; every DMA below is partition-major linear.
    molT = nc.dram_tensor("molT", [128, KM, NS], F32R, kind="ExternalInput").ap()
    protT = nc.dram_tensor("protT", [128, KP, MS], F32R, kind="ExternalInput").ap()
    wm1 = nc.dram_tensor("wm1", [DC, 128, KM * 128], F32R, kind="ExternalInput").ap()
    wm2 = nc.dram_tensor("wm2", [DC, 128, KD * 128], F32R, kind="ExternalInput").ap()
    wp1 = nc.dram_tensor("wp1", [DC, 128, KP * 128], F32R, kind="ExternalInput").ap()
    wp2 = nc.dram_tensor("wp2", [DC, 128, KD * 128], F32R, kind="ExternalInput").ap()
    bm1 = nc.dram_tensor("bm1", [128, DC], F32, kind="ExternalInput").ap()
    bm2 = nc.dram_tensor("bm2", [128, DC], F32, kind="ExternalInput").ap()
    bp1 = nc.dram_tensor("bp1", [128, DC], F32, kind="ExternalInput").ap()
    bp2 = nc.dram_tensor("bp2", [128, DC], F32, kind="ExternalInput").ap()
    invtemp = nc.dram_tensor("invtemp", [1, 1], F32, kind="ExternalInput").ap()
    ones_d = nc.dram_tensor("ones", [128, 128], F32R, kind="ExternalInput").ap()
    S = nc.dram_tensor("S", [N_CORES, DC, 128, NS], F32, kind="ExternalOutput").ap()

    with tile.TileContext(nc) as tc, ExitStack() as ctx, \
            nc.allow_low_precision(reason="float32r tiles are bit-identical fp32"):
        dram = ctx.enter_context(tc.tile_pool(name="dram", bufs=1, space="DRAM"))
        send = dram.tile([128, DC, NS], BF16)            # Mn shard, partition-major
        recv = dram.tile([N_CORES, 128, DC, NS], BF16, addr_space="Shared")

        sb = ctx.enter_context(tc.tile_pool(name="sb", bufs=1))
        wstream = ctx.enter_context(tc.tile_pool(name="w", bufs=4))
        mn_pool = ctx.enter_context(tc.tile_pool(name="mn", bufs=2))
        st_pool = ctx.enter_context(tc.tile_pool(name="st", bufs=4))
        ps = ctx.enter_context(tc.tile_pool(name="ps", bufs=4, space="PSUM"))
        psn = ctx.enter_context(tc.tile_pool(name="psn", bufs=2, space="PSUM"))
        psb = ctx.enter_context(tc.tile_pool(name="psb", bufs=2, space="PSUM"))

        # ---- constants ----
        ones_col = sb.tile([128, 1], F32R, tag="ones_col")
        nc.gpsimd.dma_start(out=ones_col[:], in_=ones_d[:, 0:1])
        ones_row = sb.tile([1, 128], F32R, tag="ones_row")
        nc.gpsimd.dma_start(out=ones_row[:], in_=ones_d[0:1, :])
        invt = sb.tile([128, 1], F32, tag="invt")
        nc.gpsimd.dma_start(out=invt[:], in_=invtemp.to_broadcast([128, 1]))

        def load_bias(name, ap):
            t = sb.tile([128, DC], F32, tag=name)
            nc.gpsimd.dma_start(out=t[:], in_=ap[:])
            return t

        bm1_s, bm2_s = load_bias("bm1", bm1), load_bias("bm2", bm2)
        bp1_s, bp2_s = load_bias("bp1", bp1), load_bias("bp2", bp2)

        def mlp_layer(x_tile, w_dram, kchunks, ncols, bias_tile, relu, out_tile,
                      dma_engine=None):
            """out[128, DC, ncols] = act(w.T @ x + b); all feature-major."""
            nhalves = ncols // 512
            for h in range(DC):
                wcol = wstream.tile([128, kchunks, 128], F32R, tag="wcol")
                (dma_engine or nc.sync).dma_start(
                    out=wcol[:],
                    in_=w_dram[h].rearrange("p (k m) -> p k m", k=kchunks))
                for nh in range(nhalves):
                    pt = ps.tile([128, 512], F32, tag="mm")
                    for k in range(kchunks):
                        nc.tensor.matmul(
                            pt[:], wcol[:, k, :],
                            x_tile[:, k, nh * 512:(nh + 1) * 512],
                            start=(k == 0), stop=(k == kchunks - 1),
                        )
                    nc.scalar.activation(
                        out_tile[:, h, nh * 512:(nh + 1) * 512], pt[:],
                        AF.Relu if relu else AF.Identity,
                        bias=bias_tile[:, h:h + 1],
                    )

        def normalize_cols(x_tile, lo, width, out_tile, send_dram):
            """L2-normalize columns [lo, lo+width) of x [128, DC, *] into bf16
            out_tile and stream each d-chunk to send_dram as it is scaled."""
            pn = psn.tile([1, width], F32, tag="psn")
            for k in range(DC):
                sq = st_pool.tile([128, width], F32R, tag="sq")
                nc.vector.tensor_mul(sq[:], x_tile[:, k, lo:lo + width],
                                     x_tile[:, k, lo:lo + width])
                nc.tensor.matmul(pn[:], ones_col[:], sq[:],
                                 start=(k == 0), stop=(k == DC - 1))
            nsq = sb.tile([1, width], F32, tag="normsq")
            nc.scalar.activation(nsq[:], pn[:], AF.Sqrt)
            nc.vector.tensor_scalar_max(nsq[:], nsq[:], EPS)
            inv = sb.tile([1, width], F32R, tag="invn")
            nc.vector.reciprocal(inv[:], nsq[:])
            binv = sb.tile([128, width], F32, tag="binv")
            pb = psb.tile([128, width], F32, tag="psb")
            nc.tensor.matmul(pb[:], ones_row[:], inv[:], start=True, stop=True)
            nc.vector.tensor_copy(binv[:], pb[:])
            for k in range(DC):
                nc.vector.tensor_mul(out_tile[:, k, lo:lo + width],
                                     x_tile[:, k, lo:lo + width], binv[:])
                nc.gpsimd.dma_start(out=send_dram[:, k, :],
                                    in_=out_tile[:, k, lo:lo + width])

        # ================= molecule MLP (N shard) =================
        molT_s = sb.tile([128, KM, NS], F32R, tag="molT")
        nc.sync.dma_start(out=molT_s[:, 0:KM // 2, :], in_=molT[:, 0:KM // 2, :])
        nc.scalar.dma_start(out=molT_s[:, KM // 2:, :], in_=molT[:, KM // 2:, :])
        Hm = sb.tile([128, DC, NS], F32R, tag="hid")
        mlp_layer(molT_s, wm1, KM, NS, bm1_s, True, Hm)
        Mp = sb.tile([128, DC, NS], F32R, tag="emb")
        mlp_layer(Hm, wm2, KD, NS, bm2_s, False, Mp)
        Mnb = sb.tile([128, DC, NS], BF16, tag="mnb")
        normalize_cols(Mp, 0, NS, Mnb, send)

        # ================= AllGather molecule embeddings =================
        nc.gpsimd.collective_compute(
            "AllGather",
            mybir.AluOpType.bypass,
            replica_groups=[list(range(N_CORES))],
            ins=[send[:]],
            outs=[recv[:]],
        )

        # ================= protein MLP (M shard) =================
        protT_s = sb.tile([128, KP, MS], F32R, tag="protT")
        nc.scalar.dma_start(out=protT_s[:], in_=protT[:])
        Hp = sb.tile([128, DC, MS], F32R, tag="hid")
        mlp_layer(protT_s, wp1, KP, MS, bp1_s, True, Hp,
                  dma_engine=nc.scalar)
        Pp = sb.tile([128, DC, MS], F32R, tag="emb")
        mlp_layer(Hp, wp2, KD, MS, bp2_s, False, Pp)
        # bf16 copy of raw Pp, chunk-by-chunk so the casts overlap layer 2
        # (normalization is folded into the S-tile eviction scale instead).
        Ppb = sb.tile([128, DC, MS], BF16, tag="mnb")
        for k in range(DC):
            nc.vector.tensor_copy(Ppb[:, k, :], Pp[:, k, :])
        # row-form |pp|^2 -> [1, MS]
        nsq_p = sb.tile([1, MS], F32, tag="normsq_p")
        for nh in range(MS // 512):
            pn = psn.tile([1, 512], F32, tag="psn")
            for k in range(DC):
                sq = st_pool.tile([128, 512], F32R, tag="sq")
                nc.vector.tensor_mul(
                    sq[:], Pp[:, k, nh * 512:(nh + 1) * 512],
                    Pp[:, k, nh * 512:(nh + 1) * 512])
                nc.tensor.matmul(pn[:], ones_col[:], sq[:],
                                 start=(k == 0), stop=(k == DC - 1))
            nc.scalar.activation(nsq_p[:, nh * 512:(nh + 1) * 512], pn[:],
                                 AF.Sqrt)
        nc.vector.tensor_scalar_max(nsq_p[:], nsq_p[:], EPS)
        inv_p = sb.tile([1, MS], F32, tag="invn_p")
        nc.vector.reciprocal(inv_p[:], nsq_p[:])
        # transpose [1, MS] -> column-form [128, DC] via 8 outer-product mms
        ones_f32 = sb.tile([1, 1], F32, tag="ones_f32")
        nc.scalar.activation(ones_f32[:], ones_col[0:1, 0:1], AF.Copy)
        pcol = psb.tile([128, DC], F32, tag="psb")
        for j in range(DC):
            nc.tensor.matmul(pcol[:, j:j + 1], inv_p[0:1, j * 128:(j + 1) * 128],
                             ones_f32[0:1, 0:1], start=(j == 0), stop=(j == DC - 1))
        scale_col = sb.tile([128, DC], F32, tag="scale_col")
        nc.scalar.activation(scale_col[:], pcol[:], AF.Copy)
        nc.vector.tensor_scalar_mul(scale_col[:], scale_col[:], invt[:, 0:1])

        # ================= similarity tiles =================
        for c in range(N_CORES):
            mnb = mn_pool.tile([128, DC, NS], BF16, tag="mn")
            nc.sync.dma_start(out=mnb[:], in_=recv[c])
            for mi in range(MS // 128):
                pt = ps.tile([128, 512], F32, tag="mm")
                for k in range(KD):
                    nc.tensor.matmul(
                        pt[:], Ppb[:, k, mi * 128:(mi + 1) * 128],
                        mnb[:, k, :],
                        start=(k == 0), stop=(k == KD - 1),
                    )
                stile = st_pool.tile([128, NS], F32, tag="stile")
                nc.scalar.activation(stile[:], pt[:], AF.Copy,
                                     scale=scale_col[:, mi:mi + 1])
                nc.gpsimd.dma_start(out=S[c, mi], in_=stile[:])

    nc.compile()
    _CACHE["nc"] = nc
    return nc


def _tile_w(W):
    """W [D, K] (fp32) -> [DC, 128, K] where element (h, p, k*128+m) =
    W[h*128+m, k*128+p]: per-h slab is a linear partition-major wcol load."""
    Dout, K = W.shape
    kc = K // 128
    t = W.reshape(DC, 128, kc, 128).transpose(0, 3, 2, 1)   # [h, p, k, m]
    return np.ascontiguousarray(t.reshape(DC, 128, kc * 128))


def _tile_x(Xshard):
    """X [rows, K] -> [128, KC, rows] feature-major partition-tiled."""
    rows, K = Xshard.shape
    kc = K // 128
    t = Xshard.reshape(rows, kc, 128).transpose(2, 1, 0)    # [p, k, rows]
    return np.ascontiguousarray(t)


def kernel(molecule, protein, Wm1, bm1, Wm2, bm2, Wp1, bp1, Wp2, bp2,
           temperature):
    nc = _build()

    molecule = np.asarray(molecule, np.float32)
    protein = np.asarray(protein, np.float32)
    wm1 = _tile_w(np.asarray(Wm1, np.float32))
    wm2 = _tile_w(np.asarray(Wm2, np.float32))
    wp1 = _tile_w(np.asarray(Wp1, np.float32))
    wp2 = _tile_w(np.asarray(Wp2, np.float32))

    def tile_b(b):
        return np.ascontiguousarray(np.asarray(b, np.float32).reshape(DC, 128).T)

    bm1_np, bm2_np = tile_b(bm1), tile_b(bm2)
    bp1_np, bp2_np = tile_b(bp1), tile_b(bp2)
    invt = (1.0 / np.asarray(temperature, np.float32)).reshape(1, 1)
    ones_np = np.ones((128, 128), np.float32)

    in_maps = []
    for c in range(N_CORES):
        in_maps.append({
            "molT": _tile_x(molecule[c * NS:(c + 1) * NS]),
            "protT": _tile_x(protein[c * MS:(c + 1) * MS]),
            "wm1": wm1, "wm2": wm2, "wp1": wp1, "wp2": wp2,
            "bm1": bm1_np, "bm2": bm2_np, "bp1": bp1_np, "bp2": bp2_np,
            "invtemp": invt, "ones": ones_np,
        })

    _CACHE["in_maps"] = in_maps
    res = run_bass_kernel_spmd(nc, in_maps, list(range(N_CORES)))
    out = np.empty((M, N), np.float32)
    for c in range(N_CORES):
        # S block layout [c2, mi, 128, 512] -> rows mi*128+i, cols c2*512+j
        blk = res.results[c]["S"]                      # [8, 8, 128, 512]
        out[c * MS:(c + 1) * MS] = blk.transpose(1, 2, 0, 3).reshape(MS, N)
    return out


# revision 17
# speedup vs baseline: 1.0209x; 1.0209x over previous
"""Trainium2 Bass kernel for nn_Coembedding (dual-MLP cosine-similarity retrieval).

Computation (see reference):
    mp = relu(molecule @ Wm1.T + bm1) @ Wm2.T + bm2          [N, D]
    pp = relu(protein  @ Wp1.T + bp1) @ Wp2.T + bp2          [M, D]
    out = (pp/|pp| @ (mp/|mp|).T) / temperature              [M, N]

Distribution over 8 NeuronCores:
  - molecule rows (N) sharded 8x for the molecule MLP; normalized embeddings
    (feature-major [D, N/8]) AllGathered so every core holds all N molecule
    embeddings.
  - protein rows (M) sharded 8x; each core computes its own protein MLP shard
    and the [M/8, N] similarity tile.

All on-chip layouts are feature-major (K on partitions) so the two MLP layers
and the similarity GEMM chain without transposes.  Matmuls run as float32r
(full-rate fp32 streaming, ~1e-4 matmul rel-err vs 4x slower exact fp32).
All DRAM layouts are pre-tiled host-side so every DMA is a linear
partition-major copy (large contiguous runs per partition).
"""

import numpy as np
from contextlib import ExitStack

import concourse.bass as bass
import concourse.tile as tile
from concourse import bacc, mybir
from concourse.bass_utils import run_bass_kernel_spmd

F32 = mybir.dt.float32
F32R = mybir.dt.float32r
BF16 = mybir.dt.bfloat16
AF = mybir.ActivationFunctionType

N_CORES = 8
N, M, MOL, PROT, D = 4096, 8192, 768, 1280, 1024
NS = N // N_CORES            # 512 molecule rows per core
MS = M // N_CORES            # 1024 protein rows per core
KM, KP, KD = MOL // 128, PROT // 128, D // 128   # 6, 10, 8 contraction chunks
DC = D // 128                # 8 output-feature chunks
EPS = 1e-8

_CACHE: dict = {}


def _build():
    if "nc" in _CACHE:
        return _CACHE["nc"]

    nc = bacc.Bacc("TRN2", target_bir_lowering=False, debug=False,
                   num_devices=N_CORES)

    # All inputs pre-tiled host-side; every DMA below is partition-major linear.
    molT = nc.dram_tensor("molT", [128, KM, NS], F32R, kind="ExternalInput").ap()
    protT = nc.dram_tensor("protT", [128, KP, MS], F32R, kind="ExternalInput").ap()
    wm1 = nc.dram_tensor("wm1", [DC, 128, KM * 128], F32R, kind="ExternalInput").ap()
    wm2 = nc.dram_tensor("wm2", [DC, 128, KD * 128], F32R, kind="ExternalInput").ap()
    wp1 = nc.dram_tensor("wp1", [DC, 128, KP * 128], F32R, kind="ExternalInput").ap()
    wp2 = nc.dram_tensor("wp2", [DC, 128, KD * 128], F32R, kind="ExternalInput").ap()
    bm1 = nc.dram_tensor("bm1", [128, DC], F32, kind="ExternalInput").ap()
    bm2 = nc.dram_tensor("bm2", [128, DC], F32, kind="ExternalInput").ap()
    bp1 = nc.dram_tensor("bp1", [128, DC], F32, kind="ExternalInput").ap()
    bp2 = nc.dram_tensor("bp2", [128, DC], F32, kind="ExternalInput").ap()
    invtemp = nc.dram_tensor("invtemp", [1, 1], F32, kind="ExternalInput").ap()
    ones_d = nc.dram_tensor("ones", [128, 128], F32R, kind="ExternalInput").ap()
    S = nc.dram_tensor("S", [N_CORES, DC, 128, NS], F32, kind="ExternalOutput").ap()

    with tile.TileContext(nc) as tc, ExitStack() as ctx, \
            nc.allow_low_precision(reason="float32r tiles are bit-identical fp32"):
        dram = ctx.enter_context(tc.tile_pool(name="dram", bufs=1, space="DRAM"))
        send = dram.tile([128, DC, NS], BF16)            # Mn shard, partition-major
        recv = dram.tile([N_CORES, 128, DC, NS], BF16, addr_space="Shared")

        sb = ctx.enter_context(tc.tile_pool(name="sb", bufs=1))
        wstream = ctx.enter_context(tc.tile_pool(name="w", bufs=4))
        mn_pool = ctx.enter_context(tc.tile_pool(name="mn", bufs=2))
        st_pool = ctx.enter_context(tc.tile_pool(name="st", bufs=4))
        ps = ctx.enter_context(tc.tile_pool(name="ps", bufs=4, space="PSUM"))
        psn = ctx.enter_context(tc.tile_pool(name="psn", bufs=2, space="PSUM"))
        psb = ctx.enter_context(tc.tile_pool(name="psb", bufs=2, space="PSUM"))

        # ---- constants ----
        ones_col = sb.tile([128, 1], F32R, tag="ones_col")
        nc.gpsimd.dma_start(out=ones_col[:], in_=ones_d[:, 0:1])
        ones_row = sb.tile([1, 128], F32R, tag="ones_row")
        nc.gpsimd.dma_start(out=ones_row[:], in_=ones_d[0:1, :])
        invt = sb.tile([128, 1], F32, tag="invt")
        nc.gpsimd.dma_start(out=invt[:], in_=invtemp.to_broadcast([128, 1]))

        def load_bias(name, ap):
            t = sb.tile([128, DC], F32, tag=name)
            nc.gpsimd.dma_start(out=t[:], in_=ap[:])
            return t

        bm1_s, bm2_s = load_bias("bm1", bm1), load_bias("bm2", bm2)
        bp1_s, bp2_s = load_bias("bp1", bp1), load_bias("bp2", bp2)

        def mlp_layer(x_tile, w_dram, kchunks, ncols, bias_tile, relu, out_tile,
                      dma_engine=None):
            """out[128, DC, ncols] = act(w.T @ x + b); all feature-major."""
            nhalves = ncols // 512
            for h in range(DC):
                wcol = wstream.tile([128, kchunks, 128], F32R, tag="wcol")
                (dma_engine or nc.sync).dma_start(
                    out=wcol[:],
                    in_=w_dram[h].rearrange("p (k m) -> p k m", k=kchunks))
                for nh in range(nhalves):
                    pt = ps.tile([128, 512], F32, tag="mm")
                    for k in range(kchunks):
                        nc.tensor.matmul(
                            pt[:], wcol[:, k, :],
                            x_tile[:, k, nh * 512:(nh + 1) * 512],
                            start=(k == 0), stop=(k == kchunks - 1),
                        )
                    nc.scalar.activation(
                        out_tile[:, h, nh * 512:(nh + 1) * 512], pt[:],
                        AF.Relu if relu else AF.Identity,
                        bias=bias_tile[:, h:h + 1],
                    )

        def normalize_cols(x_tile, lo, width, out_tile, send_dram):
            """L2-normalize columns [lo, lo+width) of x [128, DC, *] into bf16
            out_tile and stream each d-chunk to send_dram as it is scaled."""
            pn = psn.tile([1, width], F32, tag="psn")
            for k in range(DC):
                sq = st_pool.tile([128, width], F32R, tag="sq", bufs=8)
                nc.vector.tensor_mul(sq[:], x_tile[:, k, lo:lo + width],
                                     x_tile[:, k, lo:lo + width])
                nc.tensor.matmul(pn[:], ones_col[:], sq[:],
                                 start=(k == 0), stop=(k == DC - 1))
            nsq = sb.tile([1, width], F32, tag="normsq")
            nc.scalar.activation(nsq[:], pn[:], AF.Sqrt)
            nc.vector.tensor_scalar_max(nsq[:], nsq[:], EPS)
            inv = sb.tile([1, width], F32R, tag="invn")
            nc.vector.reciprocal(inv[:], nsq[:])
            binv = sb.tile([128, width], F32, tag="binv")
            pb = psb.tile([128, width], F32, tag="psb")
            nc.tensor.matmul(pb[:], ones_row[:], inv[:], start=True, stop=True)
            nc.vector.tensor_copy(binv[:], pb[:])
            for k in range(DC):
                nc.vector.tensor_mul(out_tile[:, k, lo:lo + width],
                                     x_tile[:, k, lo:lo + width], binv[:])
                nc.gpsimd.dma_start(out=send_dram[:, k, :],
                                    in_=out_tile[:, k, lo:lo + width])

        # ================= molecule MLP (N shard) =================
        molT_s = sb.tile([128, KM, NS], F32R, tag="molT")
        nc.sync.dma_start(out=molT_s[:, 0:KM // 2, :], in_=molT[:, 0:KM // 2, :])
        nc.scalar.dma_start(out=molT_s[:, KM // 2:, :], in_=molT[:, KM // 2:, :])
        Hm = sb.tile([128, DC, NS], F32R, tag="hid")
        mlp_layer(molT_s, wm1, KM, NS, bm1_s, True, Hm)
        Mp = sb.tile([128, DC, NS], F32R, tag="emb")
        mlp_layer(Hm, wm2, KD, NS, bm2_s, False, Mp)
        Mnb = sb.tile([128, DC, NS], BF16, tag="mnb")
        normalize_cols(Mp, 0, NS, Mnb, send)

        # ================= AllGather molecule embeddings =================
        nc.gpsimd.collective_compute(
            "AllGather",
            mybir.AluOpType.bypass,
            replica_groups=[list(range(N_CORES))],
            ins=[send[:]],
            outs=[recv[:]],
        )

        # ================= protein MLP (M shard) =================
        protT_s = sb.tile([128, KP, MS], F32R, tag="protT")
        nc.scalar.dma_start(out=protT_s[:], in_=protT[:])
        Hp = sb.tile([128, DC, MS], F32R, tag="hid")
        mlp_layer(protT_s, wp1, KP, MS, bp1_s, True, Hp,
                  dma_engine=nc.scalar)
        Pp = sb.tile([128, DC, MS], F32R, tag="emb")
        mlp_layer(Hp, wp2, KD, MS, bp2_s, False, Pp)
        # bf16 copy of raw Pp, chunk-by-chunk so the casts overlap layer 2
        # (normalization is folded into the S-tile eviction scale instead).
        Ppb = sb.tile([128, DC, MS], BF16, tag="mnb")
        for k in range(DC):
            nc.vector.tensor_copy(Ppb[:, k, :], Pp[:, k, :])
        # row-form |pp|^2 -> [1, MS]
        nsq_p = sb.tile([1, MS], F32, tag="normsq_p")
        for nh in range(MS // 512):
            pn = psn.tile([1, 512], F32, tag="psn")
            for k in range(DC):
                sq = st_pool.tile([128, 512], F32R, tag="sq", bufs=8)
                nc.vector.tensor_mul(
                    sq[:], Pp[:, k, nh * 512:(nh + 1) * 512],
                    Pp[:, k, nh * 512:(nh + 1) * 512])
                nc.tensor.matmul(pn[:], ones_col[:], sq[:],
                                 start=(k == 0), stop=(k == DC - 1))
            nc.scalar.activation(nsq_p[:, nh * 512:(nh + 1) * 512], pn[:],
                                 AF.Sqrt)
        nc.vector.tensor_scalar_max(nsq_p[:], nsq_p[:], EPS)
        inv_p = sb.tile([1, MS], F32, tag="invn_p")
        nc.vector.reciprocal(inv_p[:], nsq_p[:])
        # transpose [1, MS] -> column-form [128, DC] via 8 outer-product mms
        ones_f32 = sb.tile([1, 1], F32, tag="ones_f32")
        nc.scalar.activation(ones_f32[:], ones_col[0:1, 0:1], AF.Copy)
        pcol = psb.tile([128, DC], F32, tag="psb")
        for j in range(DC):
            nc.tensor.matmul(pcol[:, j:j + 1], inv_p[0:1, j * 128:(j + 1) * 128],
                             ones_f32[0:1, 0:1], start=(j == 0), stop=(j == DC - 1))
        scale_col = sb.tile([128, DC], F32, tag="scale_col")
        nc.scalar.activation(scale_col[:], pcol[:], AF.Copy)
        nc.vector.tensor_scalar_mul(scale_col[:], scale_col[:], invt[:, 0:1])

        # ================= similarity tiles =================
        for c in range(N_CORES):
            mnb = mn_pool.tile([128, DC, NS], BF16, tag="mn")
            nc.sync.dma_start(out=mnb[:], in_=recv[c])
            for mi in range(MS // 128):
                pt = ps.tile([128, 512], F32, tag="mm")
                for k in range(KD):
                    nc.tensor.matmul(
                        pt[:], Ppb[:, k, mi * 128:(mi + 1) * 128],
                        mnb[:, k, :],
                        start=(k == 0), stop=(k == KD - 1),
                    )
                stile = st_pool.tile([128, NS], F32, tag="stile")
                nc.scalar.activation(stile[:], pt[:], AF.Copy,
                                     scale=scale_col[:, mi:mi + 1])
                nc.gpsimd.dma_start(out=S[c, mi], in_=stile[:])

    nc.compile()
    _CACHE["nc"] = nc
    return nc


def _tile_w(W):
    """W [D, K] (fp32) -> [DC, 128, K] where element (h, p, k*128+m) =
    W[h*128+m, k*128+p]: per-h slab is a linear partition-major wcol load."""
    Dout, K = W.shape
    kc = K // 128
    t = W.reshape(DC, 128, kc, 128).transpose(0, 3, 2, 1)   # [h, p, k, m]
    return np.ascontiguousarray(t.reshape(DC, 128, kc * 128))


def _tile_x(Xshard):
    """X [rows, K] -> [128, KC, rows] feature-major partition-tiled."""
    rows, K = Xshard.shape
    kc = K // 128
    t = Xshard.reshape(rows, kc, 128).transpose(2, 1, 0)    # [p, k, rows]
    return np.ascontiguousarray(t)


def kernel(molecule, protein, Wm1, bm1, Wm2, bm2, Wp1, bp1, Wp2, bp2,
           temperature):
    nc = _build()

    molecule = np.asarray(molecule, np.float32)
    protein = np.asarray(protein, np.float32)
    wm1 = _tile_w(np.asarray(Wm1, np.float32))
    wm2 = _tile_w(np.asarray(Wm2, np.float32))
    wp1 = _tile_w(np.asarray(Wp1, np.float32))
    wp2 = _tile_w(np.asarray(Wp2, np.float32))

    def tile_b(b):
        return np.ascontiguousarray(np.asarray(b, np.float32).reshape(DC, 128).T)

    bm1_np, bm2_np = tile_b(bm1), tile_b(bm2)
    bp1_np, bp2_np = tile_b(bp1), tile_b(bp2)
    invt = (1.0 / np.asarray(temperature, np.float32)).reshape(1, 1)
    ones_np = np.ones((128, 128), np.float32)

    in_maps = []
    for c in range(N_CORES):
        in_maps.append({
            "molT": _tile_x(molecule[c * NS:(c + 1) * NS]),
            "protT": _tile_x(protein[c * MS:(c + 1) * MS]),
            "wm1": wm1, "wm2": wm2, "wp1": wp1, "wp2": wp2,
            "bm1": bm1_np, "bm2": bm2_np, "bp1": bp1_np, "bp2": bp2_np,
            "invtemp": invt, "ones": ones_np,
        })

    _CACHE["in_maps"] = in_maps
    res = run_bass_kernel_spmd(nc, in_maps, list(range(N_CORES)))
    out = np.empty((M, N), np.float32)
    for c in range(N_CORES):
        # S block layout [c2, mi, 128, 512] -> rows mi*128+i, cols c2*512+j
        blk = res.results[c]["S"]                      # [8, 8, 128, 512]
        out[c * MS:(c + 1) * MS] = blk.transpose(1, 2, 0, 3).reshape(MS, N)
    return out
